# revision 41
# baseline (speedup 1.0000x reference)
"""MGU (minimal gated unit) Bass kernel for Trainium2, 8-core SPMD.

Problem: B=128, T=512, D=U=512 fp32.
    xf = x @ Wf + bf ; xh = x @ Wh + bh            (parallel over B,T)
    scan over t: f = sigmoid(xf_t + h @ Uf)
                 S = tanh(xh_t + (f*h) @ Uh)
                 h = (1-f)*h + f*S
Output: final h [B, U].

Sharding: data-parallel over B (16 rows/core), weights replicated.

Layout ("T-layout"): U (or D) stays on the partition axis, batch on the
free axis, so the sequential recurrence needs no per-step transposes:
  - h/f/S/g tiles: [128p, kt*16b] = [128, 64]   (kt = U/128 = 4)
  - per-step matmul zT[m] = sum_k Uf[k,m].T @ hT[k] -> [128, 4*16] PSUM
Matmuls in bf16 (fp32 PSUM accumulate), gates/state bf16; ~8e-3 max
rel err vs the fp32 reference.

The scan is latency-bound: each step is a serial cross-engine chain
(PE matmuls -> ACT sigmoid -> DVE mult -> PE matmuls -> ACT tanh -> DVE
mult), ~2.6us/step. Structure of the chain, in dependency order, and
the tricks that shorten it:
  - xf_t/xh_t are seeded into the PSUM accumulator by an identity-weight
    matmul (sets has_written), removing DVE adds from the serial chain;
    sigmoid/tanh read PSUM directly.
  - h' = t2 + t3 (t2 = h - f*h on GpSimd, t3 = f*S on DVE) is summed on
    the idle GpSimd engine OFF the chain; the next step's zf PSUM
    accumulates seed + t2@Uf + t3@Uf as separate moving operands, so
    only the t3-part matmuls (16) wait on the chain's last DVE op.
  - The t3-part matmuls are k-split: the k01 half issues as soon as
    t3's low half exists; t3 itself is computed in u-halves so its low
    half exists right after tanh's low half.
  - BOTH gate PSUMs (zf and zh) are split into lo/hi tiles (m01/m23):
    PSUM reads are tile-granular in the dependency tracker, so the split
    lets each activation start as soon as its own tile's writers drain.
    sigmoid and tanh are each two ACTs (lo then hi, which the ACT engine
    pipelines with ~60ns overlap); g and t3 are two DVE ops. The zh
    matmuls are k-subgrouped so the k01 contraction consumes g's low
    half while sigmoid_hi/g_hi still compute — without the tile split
    this cascade is strictly slower (the serialized second ACT costs
    more than the early start buys; measured both ways).
  - Gates f/S and state h/t2/t3 are bf16: all-16-bit DVE ops run in the
    2x path (~190ns vs ~225ns).
  - Phase-1 projection matmuls are interleaved into the scan's PE idle
    windows (a 2-matmul half-group every other step, emitted at the END
    of the body so the proj ACT queues behind tanh_hi, not before it);
    chunk c+1's projections run during chunk c, and chunk 1's land in
    chunk 0's even slots, keeping the per-step load flat from step 0.
Known non-fixable costs: ~28us prologue (weight/x DMA + chunk-0
projections) and ~200us of PE clock ramp-up (the first ~60 steps run
~3.1us until the clock boosts).

Measurement gotcha: the PE clock can be STUCK at ~2.0GHz from a prior
process (+20% runtime); NEURON_RT_RESET_CORES=1 (set below) restores
2.4GHz. Without it, back-to-back runs are not comparable.
"""

import os

# Reset cores on runtime init: the PE clock can be stuck in a degraded
# p-state from a previous process; a reset restores the full 2.4 GHz
# (~20% kernel-time difference). Must be set before the NRT initializes.
os.environ.setdefault("NEURON_RT_RESET_CORES", "1")

import numpy as np
import ml_dtypes

import concourse.bass as bass
import concourse.bacc as bacc
import concourse.mybir as mybir
from concourse import tile
from concourse.bass_utils import run_bass_kernel_spmd

B, T, D, U = 128, 512, 512, 512
NCORES = 8
BC = B // NCORES          # batch rows per core = 16
KT = D // 128             # 4 contraction tiles
MT = U // 128             # 4 output tiles
CHUNK = 32                # phase-1 time-chunk; N = CHUNK*BC = 512 per matmul
GW = MT * BC              # scan tile width = 64

BF16 = mybir.dt.bfloat16
F32 = mybir.dt.float32
NPBF16 = ml_dtypes.bfloat16
AF = mybir.ActivationFunctionType
ALU = mybir.AluOpType

_CACHE = {}
LAST_RESULTS = None  # test harness reads exec_time_ns / profile from here


def _build(t_steps: int):
    nc = bacc.Bacc("TRN2", target_bir_lowering=False, debug=False)
    nchunk = (t_steps + CHUNK - 1) // CHUNK

    x_d = nc.dram_tensor("xT", [KT, 128, T * BC], BF16, kind="ExternalInput")
    wf_d = nc.dram_tensor("WfT", [128, KT * U], BF16, kind="ExternalInput")
    wh_d = nc.dram_tensor("WhT", [128, KT * U], BF16, kind="ExternalInput")
    uf_d = nc.dram_tensor("UfT", [128, KT * U], BF16, kind="ExternalInput")
    uh_d = nc.dram_tensor("UhT", [128, KT * U], BF16, kind="ExternalInput")
    bf_d = nc.dram_tensor("bfT", [128, MT], F32, kind="ExternalInput")
    bh_d = nc.dram_tensor("bhT", [128, MT], F32, kind="ExternalInput")
    eye_d = nc.dram_tensor("eye", [128, 128], BF16, kind="ExternalInput")
    out_d = nc.dram_tensor("hT_out", [128, KT * BC], F32, kind="ExternalOutput")

    with tile.TileContext(nc) as tc:
        with (
            tc.tile_pool(name="const", bufs=1) as cpool,
            tc.tile_pool(name="xchunk", bufs=3) as xpool,
            tc.tile_pool(name="proj", bufs=16) as projpool,
            tc.tile_pool(name="work", bufs=4) as wpool,
            tc.tile_pool(name="spsum", bufs=2, space="PSUM") as spsum,
            tc.tile_pool(name="zpsum", bufs=1, space="PSUM") as zpsum,
            tc.tile_pool(name="ppsum", bufs=2, space="PSUM") as ppsum,
        ):
            # ---- resident tensors ----
            wf_sb = cpool.tile([128, KT * U], BF16, tag="wf")
            wh_sb = cpool.tile([128, KT * U], BF16, tag="wh")
            uf_sb = cpool.tile([128, KT * U], BF16, tag="uf")
            uh_sb = cpool.tile([128, KT * U], BF16, tag="uh")
            bf_sb = cpool.tile([128, MT], F32, tag="bf")
            bh_sb = cpool.tile([128, MT], F32, tag="bh")
            eye_sb = cpool.tile([128, 128], BF16, tag="eye")

            # DMA order matters for the prologue: the first projection
            # matmuls need only Wf/Wh + chunk-0 x; the scan weights Uf/Uh
            # aren't read until ~30us in, so they stream last.
            nc.sync.dma_start(wf_sb[:], wf_d[:])
            nc.sync.dma_start(wh_sb[:], wh_d[:])
            nc.sync.dma_start(bf_sb[:], bf_d[:])
            nc.sync.dma_start(bh_sb[:], bh_d[:])
            nc.sync.dma_start(eye_sb[:], eye_d[:])

            # per-chunk projection tiles (bf16): free = (t_local, m, b)
            xf_c = [None] * nchunk
            xh_c = [None] * nchunk
            xc_c = [None] * nchunk

            def emit_chunk_dma(c):
                xc = xpool.tile([128, KT * CHUNK * BC], BF16, tag="xc")
                for k in range(KT):
                    nc.sync.dma_start(
                        xc[:, k * CHUNK * BC:(k + 1) * CHUNK * BC],
                        x_d[k, :, c * CHUNK * BC:(c + 1) * CHUNK * BC],
                    )
                xc_c[c] = xc
                xf_c[c] = projpool.tile([128, CHUNK * GW], BF16, tag="xfc", name=f"xfc{c}")
                xh_c[c] = projpool.tile([128, CHUNK * GW], BF16, tag="xhc", name=f"xhc{c}")

            proj_ps = {}  # per-chunk in-flight proj psum (two streams interleave)

            def emit_proj_half(c, gi, half, pool_copy=False):
                """Half of one (gate, m) projection group of chunk c: 2 matmuls;
                on the second half, the copy-with-bias (ACT engine normally;
                GpSimd for prologue groups, whose copies would otherwise queue
                ahead of the scan's chain-critical sigmoid/tanh and drain
                through the first ~60 steps as an ACT backlog)."""
                gate, m = divmod(gi, MT)
                w_sb, b_sb, dst = ((wf_sb, bf_sb, xf_c[c]), (wh_sb, bh_sb, xh_c[c]))[gate]
                xc = xc_c[c]
                if half == 0:
                    ps = ppsum.tile([128, CHUNK * BC], F32, tag="pp", name=f"pp{c}")
                    proj_ps[c] = ps
                ps = proj_ps[c]
                kk = (0, 1) if half == 0 else (2, 3)
                for k in kk:
                    nc.tensor.matmul(
                        ps[:],
                        w_sb[:, k * U + m * 128: k * U + (m + 1) * 128],
                        xc[:, k * CHUNK * BC:(k + 1) * CHUNK * BC],
                        start=(k == 0), stop=(k == KT - 1),
                    )
                if half == 1:
                    dv = dst[:].rearrange("p (t m b) -> p t m b", t=CHUNK, m=MT, b=BC)
                    pv = ps[:].rearrange("p (t b) -> p t b", t=CHUNK, b=BC)
                    if pool_copy:
                        nc.vector.tensor_scalar_add(dv[:, :, m, :], pv, b_sb[:, m:m + 1])
                    else:
                        nc.scalar.activation(
                            dv[:, :, m, :], pv, AF.Identity, bias=b_sb[:, m:m + 1],
                        )

            def emit_proj_group(c, gi, pool_copy=False):
                emit_proj_half(c, gi, 0, pool_copy)
                emit_proj_half(c, gi, 1, pool_copy)

            # prologue: chunk 0's projections only (step 0 needs them); chunk
            # 1's are interleaved into chunk 0's even step slots below, so the
            # scan isn't stuck behind a PE backlog for its first ~40 steps
            emit_chunk_dma(0)
            if nchunk > 1:
                emit_chunk_dma(1)
            nc.sync.dma_start(uf_sb[:], uf_d[:])
            nc.sync.dma_start(uh_sb[:], uh_d[:])
            for gi in range(2 * MT):
                emit_proj_group(0, gi, pool_copy=True)

            # ---- the sequential scan, with projection work interleaved ----
            # Critical-chain restructure: h' = t2 + t3 is computed on the idle
            # GpSimd engine OFF the serial chain; the next step's zf PSUM is
            # accumulated from the two addends separately (zf' = seed(xf') +
            # t2@Uf + t3@Uf), so only the t3-part matmuls sit on the chain.
            h = wpool.tile([128, GW], BF16, tag="h")
            nc.vector.memset(h[:], 0.0)

            def u_matmuls(z, u_sb, rhs, stop):
                for m in range(MT):
                    for k in range(KT):
                        nc.tensor.matmul(
                            z[:, m * BC:(m + 1) * BC],
                            u_sb[:, k * U + m * 128: k * U + (m + 1) * 128],
                            rhs[:, k * BC:(k + 1) * BC],
                            start=False, stop=(stop and m == MT - 1 and k == KT - 1),
                            skip_group_check=True,
                        )

            def u_matmuls_ksplit(z, u_sb, rhs, stop):
                # k-half subgroups (k01 then k23) so each rhs half is consumed
                # as soon as the half-split producer finishes; m-major within a
                # subgroup so the m01 output slice drains early for the
                # half-split activation that reads it.
                for kh in range(2):
                    for m in range(MT):
                        for k in (2 * kh, 2 * kh + 1):
                            nc.tensor.matmul(
                                z[:, m * BC:(m + 1) * BC],
                                u_sb[:, k * U + m * 128: k * U + (m + 1) * 128],
                                rhs[:, k * BC:(k + 1) * BC],
                                start=False,
                                stop=(stop and kh == 1 and m == MT - 1 and k == 3),
                                skip_group_check=True,
                            )

            def seed(z, xsrc, stop=False):
                nc.tensor.matmul(z[:], eye_sb[:], xsrc, start=True, stop=stop,
                                 skip_group_check=True)

            # zf for t=0: seed only (h_0 = 0, so no U-matmul parts).
            # zf is split lo/hi like zh so sigmoid can be two ACTs, the first
            # starting as soon as the lo tile's own writers drain.
            zf_lo = spsum.tile([128, GW // 2], F32, tag="zfl")
            zf_hi = spsum.tile([128, GW // 2], F32, tag="zfh")
            seed(zf_lo, xf_c[0][:, 0:GW // 2], stop=True)
            seed(zf_hi, xf_c[0][:, GW // 2:GW], stop=True)

            for t in range(t_steps):
                c, tl = divmod(t, CHUNK)
                last = (t == t_steps - 1)

                HF = GW // 2
                f = wpool.tile([128, GW], BF16, tag="f")
                nc.scalar.activation(f[:, 0:HF], zf_lo[:], AF.Sigmoid)
                nc.scalar.activation(f[:, HF:GW], zf_hi[:], AF.Sigmoid)
                g = wpool.tile([128, GW], BF16, tag="g")
                nc.vector.tensor_tensor(g[:, 0:HF], f[:, 0:HF], h[:, 0:HF], ALU.mult)
                nc.vector.tensor_tensor(g[:, HF:GW], f[:, HF:GW], h[:, HF:GW], ALU.mult)
                t2 = wpool.tile([128, GW], BF16, tag="t2")
                nc.gpsimd.tensor_tensor(t2[:], h[:], g[:], ALU.subtract)

                # zh as two PSUM tiles (separate accumulation groups): PSUM
                # reads are tile-granular in the dependency tracker, so
                # tanh_L's wait covers only the 8 m01 matmuls, not all 16.
                zh_lo = zpsum.tile([128, HF], F32, tag="zhl")
                zh_hi = zpsum.tile([128, HF], F32, tag="zhh")
                seed(zh_lo, xh_c[c][:, tl * GW:tl * GW + HF])
                seed(zh_hi, xh_c[c][:, tl * GW + HF:(tl + 1) * GW])
                for kh in range(2):
                    for m in range(MT):
                        z, mo = (zh_lo, m) if m < 2 else (zh_hi, m - 2)
                        for k in (2 * kh, 2 * kh + 1):
                            nc.tensor.matmul(
                                z[:, mo * BC:(mo + 1) * BC],
                                uh_sb[:, k * U + m * 128: k * U + (m + 1) * 128],
                                g[:, k * BC:(k + 1) * BC],
                                start=False,
                                stop=(kh == 1 and m % 2 == 1 and k == 2 * kh + 1),
                                skip_group_check=True,
                            )
                s = wpool.tile([128, GW], BF16, tag="s")
                nc.scalar.activation(s[:, 0:HF], zh_lo[:], AF.Tanh)
                nc.scalar.activation(s[:, HF:GW], zh_hi[:], AF.Tanh)

                t3 = wpool.tile([128, GW], F32 if last else BF16, tag="t3")
                nc.vector.tensor_tensor(t3[:, 0:HF], f[:, 0:HF], s[:, 0:HF], ALU.mult)
                nc.vector.tensor_tensor(t3[:, HF:GW], f[:, HF:GW], s[:, HF:GW], ALU.mult)
                # h' = t2 + t3 on GpSimd, off the serial chain
                hn = wpool.tile([128, GW], F32 if last else BF16, tag="hout" if last else "h")
                nc.gpsimd.tensor_tensor(hn[:], t2[:], t3[:], ALU.add)

                if not last:
                    off = ((t + 1) % CHUNK) * GW
                    xn = xf_c[(t + 1) // CHUNK]
                    zf_lo = spsum.tile([128, HF], F32, tag="zfl")
                    zf_hi = spsum.tile([128, HF], F32, tag="zfh")
                    seed(zf_lo, xn[:, off:off + HF])
                    seed(zf_hi, xn[:, off + HF:off + GW])
                    for m in range(MT):
                        z, mo = (zf_lo, m) if m < 2 else (zf_hi, m - 2)
                        for k in range(KT):
                            nc.tensor.matmul(
                                z[:, mo * BC:(mo + 1) * BC],
                                uf_sb[:, k * U + m * 128: k * U + (m + 1) * 128],
                                t2[:, k * BC:(k + 1) * BC],
                                start=False, stop=False, skip_group_check=True,
                            )
                    for kh in range(2):
                        for m in range(MT):
                            z, mo = (zf_lo, m) if m < 2 else (zf_hi, m - 2)
                            for k in (2 * kh, 2 * kh + 1):
                                nc.tensor.matmul(
                                    z[:, mo * BC:(mo + 1) * BC],
                                    uf_sb[:, k * U + m * 128: k * U + (m + 1) * 128],
                                    t3[:, k * BC:(k + 1) * BC],
                                    start=False,
                                    stop=(kh == 1 and m % 2 == 1 and k == 2 * kh + 1),
                                    skip_group_check=True,
                                )

                # projection interleave at the END of the body: the proj MMs
                # fill the PE window after the zf t3-part, and the proj ACT
                # queues after tanh_H, where the chain has ~700ns of slack
                # (mid-chain it would delay tanh and stretch the step).
                # proj schedule: chunk c+1's projections run during chunk c
                # (one half-group per odd slot — the steady-state load); chunk
                # 0 gets chunk-1's halves on its even slots so its per-step
                # load matches steady state. x DMA stays two chunks ahead.
                if c == 0 and nchunk > 1 and tl % 2 == 0 and tl < 4 * 2 * MT:
                    emit_proj_half(1, tl // 4, (tl % 4) // 2)
                if tl == 0 and c + 2 < nchunk:
                    emit_chunk_dma(c + 2)
                if c >= 1 and c + 1 < nchunk and tl % 2 == 1:
                    emit_proj_half(c + 1, (tl - 1) // 4, ((tl - 1) % 4) // 2)
                h = hn

            nc.sync.dma_start(out_d[:], h[:])

    nc.compile()
    return nc


def _prep_weight_t(w):
    # [D, U] fp32 -> [128, KT*U] bf16 with [:, k*U+m] = w[k*128+p, m]
    return np.ascontiguousarray(
        w.reshape(KT, 128, U).transpose(1, 0, 2).reshape(128, KT * U)
    ).astype(NPBF16)



def kernel(x, Wf, Uf, bf, Wh, Uh, bh):
    global LAST_RESULTS
    x = np.asarray(x, dtype=np.float32)
    Wf = np.asarray(Wf, dtype=np.float32)
    Uf = np.asarray(Uf, dtype=np.float32)
    Wh = np.asarray(Wh, dtype=np.float32)
    Uh = np.asarray(Uh, dtype=np.float32)
    bf = np.asarray(bf, dtype=np.float32)
    bh = np.asarray(bh, dtype=np.float32)

    t_steps = int(os.environ.get("BASS_MGU_T", T))
    if t_steps not in _CACHE:
        _CACHE[t_steps] = _build(t_steps)
    nc = _CACHE[t_steps]

    wf_t = _prep_weight_t(Wf)
    wh_t = _prep_weight_t(Wh)
    uf_t = _prep_weight_t(Uf)
    uh_t = _prep_weight_t(Uh)
    bf_t = np.ascontiguousarray(bf.reshape(MT, 128).T).astype(np.float32)
    bh_t = np.ascontiguousarray(bh.reshape(MT, 128).T).astype(np.float32)
    eye = np.eye(128, dtype=np.float32).astype(NPBF16)

    in_maps = []
    for ci in range(NCORES):
        xc = x[ci * BC:(ci + 1) * BC]                       # [BC, T, D]
        xt = xc.transpose(2, 1, 0)                          # [D, T, BC]
        xt = np.ascontiguousarray(xt.reshape(KT, 128, T * BC)).astype(NPBF16)
        in_maps.append({
            "xT": xt, "WfT": wf_t, "WhT": wh_t, "UfT": uf_t, "UhT": uh_t,
            "bfT": bf_t, "bhT": bh_t, "eye": eye,
        })

    trace = bool(int(os.environ.get("BASS_MGU_TRACE", "0")))
    kw = {}
    if trace and os.environ.get("BASS_TRACE_DIR"):
        kw["tmpdir"] = os.environ["BASS_TRACE_DIR"]
    res = run_bass_kernel_spmd(nc, in_maps, list(range(NCORES)), trace=trace, **kw)
    LAST_RESULTS = res

    out = np.empty((B, U), dtype=np.float32)
    for ci in range(NCORES):
        ho = np.asarray(res.results[ci]["hT_out"])          # [128, KT*BC]
        out[ci * BC:(ci + 1) * BC] = (
            ho.reshape(128, KT, BC).transpose(2, 1, 0).reshape(BC, U)
        )
    return out

